# revision 12
# baseline (speedup 1.0000x reference)
"""Trainium2 Bass kernel for CustomMamba2D.

Strategy (8 NeuronCores, data-parallel over batch; B=8 -> 1 image/core):

  Per core, one 64ch x 512x512 image is processed in 256 row-pairs
  (2 rows x 64 ch = 128 SBUF partitions).  All channel-mixing matmuls run
  with 128-partition operands using block-diagonal weights.  The depthwise
  3x3 conv is 9 PE matmuls per pair with diagonal weight blocks (3 full
  128x128 + 6 half ones, the halves packed into disjoint PE quadrants via
  tile_position).  dx shifts are free-dim AP offsets (zero pad = narrower
  accumulation range).

  The SSM decay exp(A*k) underflows to exactly 0.0 (fp32) for k >= ~1040,
  so the reference cumsum is bitwise constant from l=2048 on.  Only rows
  0..3 (l<2048) need the scan (DVE tensor_tensor_scan); every other
  position uses the constant c* = wC @ state[:,2047] per channel.

  sigmoid(g) is computed as (tanh(g/2)+1)/2 with the x0.5 folded into
  w_out, so all transcendentals (SiLU x2, tanh x2) live in ONE ACT table
  set (silu_and_others).  BN scales fold into weights host-side; BN biases
  ride the ACT activation bias operand.  Intermediates are fp16 (DVE 2x/4x
  modes); fp32 I/O; matmuls on fp32 data use float32r (1 cycle/col).
"""

import os
import sys

for _p in (
    "/root/.axon_site",
    "/root/.axon_site/_ro/trn_rl_repo",
    "/root/.axon_site/_ro/pypackages",
    "/opt/trn_rl_repo",
    "/opt/pypackages",
):
    if os.path.isdir(_p) and _p not in sys.path:
        sys.path.append(_p)

import numpy as np

import concourse.bass as bass
import concourse.mybir as mybir
from concourse.tile import TileContext
from concourse import bass_utils

f32 = mybir.dt.float32
f32r = mybir.dt.float32r
f16 = mybir.dt.float16
AF = mybir.ActivationFunctionType
ALU = mybir.AluOpType

D_MODEL, D_STATE, D_CONV, D_INNER = 64, 16, 3, 64
BN_EPS = 1e-5
B, H, W = 8, 512, 512
NP = H // 2            # 256 row pairs
HEAD_PAIRS = 2         # rows 0..3 carry the live part of the scan
HEAD_L = HEAD_PAIRS * 2 * W   # 2048
N_CORES = 8


def _split_waits(nc, maxw=1):
    """This walrus build encodes at most ONE sync wait per instruction and
    refuses to split multi-wait instructions itself.  Move extra waits onto
    NoOp carriers inserted just before the owning instruction (same engine,
    so ordering is preserved)."""
    for fn in nc.m.functions:
        for bb in fn.blocks:
            out, changed = [], False
            for inst in bb.instructions:
                si = inst.sync_info
                if si is not None and len(si.on_wait) > maxw:
                    waits = list(si.on_wait)
                    for k, wt in enumerate(waits[maxw:]):
                        out.append(
                            mybir.InstNoOp(
                                name=f"{inst.name}_sw{k}",
                                engine=inst.engine,
                                bass_nofuse=True,
                                sync_info=mybir.SyncInfo(on_wait=[wt], on_update=[]),
                            )
                        )
                    inst.sync_info = mybir.SyncInfo(
                        on_wait=waits[:maxw], on_update=list(si.on_update)
                    )
                    changed = True
                out.append(inst)
            if changed:
                bb.instructions = out


def _build_program():
    nc = bass.Bass("TRN2", target_bir_lowering=False, debug=False, num_devices=N_CORES)

    x_d = nc.dram_tensor("x", [D_MODEL, H, W], f32r, kind="ExternalInput")
    y_d = nc.dram_tensor("y", [D_MODEL, H, W], f32, kind="ExternalOutput")
    lin_d = nc.dram_tensor("lhsT_in", [128, 128], f32r, kind="ExternalInput")
    ldw_d = nc.dram_tensor("lhsT_dw", [3, 128, 128], f16, kind="ExternalInput")
    lcx_d = nc.dram_tensor("lhsT_cx", [3, 128, 64], f16, kind="ExternalInput")
    lgl_d = nc.dram_tensor("lhsT_gl", [128, 128], f16, kind="ExternalInput")
    lgh_d = nc.dram_tensor("lhsT_gh", [128, 128], f16, kind="ExternalInput")
    lo_d = nc.dram_tensor("lhsT_out", [128, 128], f16, kind="ExternalInput")
    lB_d = nc.dram_tensor("lhsT_B", [128, 16], f16, kind="ExternalInput")
    lC_d = nc.dram_tensor("lhsT_C", [16, 64], f32r, kind="ExternalInput")
    bias_d = nc.dram_tensor("biases", [128, 6], f32, kind="ExternalInput")
    dec_d = nc.dram_tensor("decay", [16, HEAD_L], f32, kind="ExternalInput")

    with TileContext(nc) as tc:
        with (
            tc.tile_pool(name="consts", bufs=1) as cpool,
            tc.tile_pool(name="xin", bufs=3) as xpool,
            tc.tile_pool(name="xp", bufs=6) as xppool,
            tc.tile_pool(name="xc", bufs=3) as xcpool,
            tc.tile_pool(name="gact", bufs=3) as gpool,
            tc.tile_pool(name="ysb", bufs=3) as ypool,
            tc.tile_pool(name="headsb", bufs=1) as hpool,
            tc.tile_pool(name="psum_in", bufs=2, space="PSUM") as pin,
            tc.tile_pool(name="psum_dw", bufs=2, space="PSUM") as pdw,
            tc.tile_pool(name="psum_g", bufs=1, space="PSUM") as pg,
            tc.tile_pool(name="psum_s", bufs=1, space="PSUM") as ps,
            tc.tile_pool(name="psum_o", bufs=1, space="PSUM") as po,
        ):
            # ---- constants into SBUF
            lin = cpool.tile([128, 128], f32r, tag="lin")
            nc.sync.dma_start(lin[:, :], lin_d[:, :])
            ldw = []
            for k in range(3):
                t = cpool.tile([128, 128], f16, tag=f"ldw{k}")
                nc.sync.dma_start(t[:, :], ldw_d[k, :, :])
                ldw.append(t)
            lcx = []
            for k in range(3):
                t = cpool.tile([128, 64], f16, tag=f"lcx{k}")
                nc.sync.dma_start(t[:, :], lcx_d[k, :, :])
                lcx.append(t)
            lgl = cpool.tile([128, 128], f16, tag="lgl")
            nc.sync.dma_start(lgl[:, :], lgl_d[:, :])
            lgh = cpool.tile([128, 128], f16, tag="lgh")
            nc.sync.dma_start(lgh[:, :], lgh_d[:, :])
            lo = cpool.tile([128, 128], f16, tag="lo")
            nc.sync.dma_start(lo[:, :], lo_d[:, :])
            lB = cpool.tile([128, 16], f16, tag="lB")
            nc.sync.dma_start(lB[:, :], lB_d[:, :])
            lC = cpool.tile([16, 64], f32r, tag="lC")
            nc.sync.dma_start(lC[:, :], lC_d[:, :])
            bias = cpool.tile([128, 6], f32, tag="bias")
            nc.sync.dma_start(bias[:, :], bias_d[:, :])
            b_in = bias[:, 0:1]
            b_conv = bias[:, 1:2]
            bg_l = bias[:, 2:3]
            bg_h = bias[:, 3:4]
            d_ch = bias[:, 4:5]
            b_out = bias[:, 5:6]

            dec = hpool.tile([16, HEAD_L], f32, tag="dec")
            nc.sync.dma_start(dec[:, :], dec_d[:, :])
            bw = hpool.tile([16, HEAD_L], f32, tag="bw")
            wsc = hpool.tile([16, HEAD_L], f32, tag="wsc")
            zer = hpool.tile([16, HEAD_L], f32, tag="zer")
            nc.gpsimd.memset(zer[:, :], 0.0)
            state = hpool.tile([16, HEAD_L], f32r, tag="state")
            spair = [
                hpool.tile([128, W], f32, tag=f"sp{j}", name=f"sp{j}")
                for j in range(HEAD_PAIRS)
            ]
            cstar = hpool.tile([128, 1], f32, tag="cstar")

            xp_tiles = {}
            xc_tiles = {}

            def load_x(j):
                t = xpool.tile([128, W], f32r, tag="x")
                nc.sync.dma_start(t[0:64, :], x_d[:, 2 * j, :])
                nc.sync.dma_start(t[64:128, :], x_d[:, 2 * j + 1, :])
                return t

            def inproj_silu(j, xt):
                p = pin.tile([128, W], f32, tag="pin")
                nc.tensor.matmul(p[:, :], lin[:, :], xt[:, :], start=True, stop=True)
                xpt = xppool.tile([128, W], f16, tag="xp")
                nc.scalar.activation(xpt[:, :], p[:, :], AF.Silu, bias=b_in, scale=1.0)
                xp_tiles[j] = xpt

            # (out_slice, in_slice) per kx = dx+1; narrower range = zero pad
            _SHIFTS = (
                (slice(1, W), slice(0, W - 1)),   # dx = -1
                (slice(0, W), slice(0, W)),       # dx = 0
                (slice(0, W - 1), slice(1, W)),   # dx = +1
            )

            def dw_silu(j):
                p = pdw.tile([128, W], f32, tag="pdw")
                tj = xp_tiles[j]
                mms = [
                    # intra-pair: ky=1 for both rows + ky=2->row2j + ky=0->row2j+1
                    dict(out=p[:, _SHIFTS[1][0]], lhsT=ldw[1][:, :], rhs=tj[:, _SHIFTS[1][1]]),
                    dict(out=p[:, _SHIFTS[0][0]], lhsT=ldw[0][:, :], rhs=tj[:, _SHIFTS[0][1]]),
                    dict(out=p[:, _SHIFTS[2][0]], lhsT=ldw[2][:, :], rhs=tj[:, _SHIFTS[2][1]]),
                ]
                if j > 0:
                    tm = xp_tiles[j - 1]   # row 2j-1 lives in parts 64:128
                    for kx in range(3):
                        osl, isl = _SHIFTS[kx]
                        mms.append(dict(
                            out=p[0:64, osl], lhsT=lcx[kx][64:128, :],
                            rhs=tm[64:128, isl], tile_position=(64, 0),
                        ))
                if j < NP - 1:
                    tp = xp_tiles[j + 1]   # row 2j+2 lives in parts 0:64
                    for kx in range(3):
                        osl, isl = _SHIFTS[kx]
                        mms.append(dict(
                            out=p[64:128, osl], lhsT=lcx[kx][0:64, :],
                            rhs=tp[0:64, isl], tile_position=(0, 64),
                        ))
                last = len(mms) - 1
                for i, mm in enumerate(mms):
                    nc.tensor.matmul(
                        mm["out"], mm["lhsT"], mm["rhs"],
                        start=(i == 0), stop=(i == last),
                        tile_position=mm.get("tile_position"),
                    )
                xct = xcpool.tile([128, W], f16, tag="xc")
                nc.scalar.activation(xct[:, :], p[:, :], AF.Silu, bias=b_conv, scale=1.0)
                xc_tiles[j] = xct

            def gating_out(j, head):
                xpt = xp_tiles[j]
                pgt = pg.tile([128, W], f32, tag="pg")
                nc.tensor.matmul(pgt[:, :], lgl[:, :], xpt[:, :], start=True, stop=True)
                tg = gpool.tile([128, W], f16, tag="tg")
                # sigmoid(z+b) = (tanh((z+b)/2)+1)/2 ; the 0.5 scale is folded
                # into w_out host-side, the +1 into the STT below.
                nc.scalar.activation(tg[:, :], pgt[:, :], AF.Tanh, bias=bg_l, scale=0.5)
                pst = ps.tile([128, W], f32, tag="ps")
                nc.tensor.matmul(pst[:, :], lgh[:, :], xpt[:, :], start=True, stop=True)
                tsh = gpool.tile([128, W], f16, tag="tsh")
                nc.scalar.activation(tsh[:, :], pst[:, :], AF.Tanh, bias=bg_h, scale=1.0)

                xct = xc_tiles.pop(j)
                sm = gpool.tile([128, W], f16, tag="sm")
                if head:
                    nc.vector.scalar_tensor_tensor(
                        sm[:, :], xct[:, :], d_ch, spair[j][:, :],
                        op0=ALU.mult, op1=ALU.add,
                    )
                else:
                    nc.vector.tensor_scalar(
                        sm[:, :], xct[:, :], d_ch, cstar[:, 0:1],
                        op0=ALU.mult, op1=ALU.add,
                    )
                u = gpool.tile([128, W], f16, tag="u")
                nc.vector.tensor_tensor(u[:, :], sm[:, :], tsh[:, :], op=ALU.add)
                g = gpool.tile([128, W], f16, tag="g")
                nc.vector.scalar_tensor_tensor(
                    g[:, :], tg[:, :], 1.0, u[:, :], op0=ALU.add, op1=ALU.mult
                )
                pot = po.tile([128, W], f32, tag="po")
                nc.tensor.matmul(pot[:, :], lo[:, :], g[:, :], start=True, stop=True)
                yt = ypool.tile([128, W], f32, tag="y")
                nc.vector.tensor_scalar(
                    yt[:, :], pot[:, :], b_out, None, op0=ALU.add
                )
                nc.sync.dma_start(y_d[:, 2 * j, :], yt[0:64, :])
                nc.sync.dma_start(y_d[:, 2 * j + 1, :], yt[64:128, :])

            # ---- head bootstrap: rows 0..3 need the real scan
            for j in range(HEAD_PAIRS + 1):
                inproj_silu(j, load_x(j))
            for j in range(HEAD_PAIRS):
                dw_silu(j)
            for r in range(2 * HEAD_PAIRS):
                j, rr = divmod(r, 2)
                pb = pg.tile([16, W], f32, tag="pg")
                nc.tensor.matmul(
                    pb[:, :], lB[64 * rr : 64 * rr + 64, :],
                    xc_tiles[j][64 * rr : 64 * rr + 64, :],
                    start=True, stop=True, tile_position=(64 * rr, 0),
                )
                nc.vector.tensor_copy(bw[:, W * r : W * (r + 1)], pb[:, :])
            nc.vector.tensor_tensor(wsc[:, :], bw[:, :], dec[:, :], op=ALU.mult)
            nc.vector.tensor_tensor_scan(
                state[:, :], zer[:, :], wsc[:, :], initial=0.0,
                op0=ALU.add, op1=ALU.add,
            )
            for r in range(2 * HEAD_PAIRS):
                j, rr = divmod(r, 2)
                pc = ps.tile([64, W], f32, tag="ps")
                nc.tensor.matmul(
                    pc[:, :], lC[:, :], state[:, W * r : W * (r + 1)],
                    start=True, stop=True,
                )
                nc.scalar.copy(spair[j][64 * rr : 64 * rr + 64, :], pc[:, :])
            # c* = (wC @ state)[:, HEAD_L-1], replicated to both halves
            nc.sync.dma_start(cstar[0:64, 0:1], spair[-1][64:128, W - 1 : W])
            nc.sync.dma_start(cstar[64:128, 0:1], spair[-1][64:128, W - 1 : W])

            for j in range(HEAD_PAIRS):
                gating_out(j, head=True)

            # ---- bulk pipeline
            for j in range(HEAD_PAIRS, NP):
                if j + 1 < NP:
                    inproj_silu(j + 1, load_x(j + 1))
                dw_silu(j)
                gating_out(j, head=False)

    _split_waits(nc, 1)
    return nc


def _prep_consts(inputs):
    fp = np.float32
    s = fp(1.0) / np.sqrt(fp(1.0) + fp(BN_EPS))

    g_in = inputs["g_in"].astype(fp); b_in = inputs["b_in"].astype(fp)
    g_conv = inputs["g_conv"].astype(fp); b_conv = inputs["b_conv"].astype(fp)
    g_gate = inputs["g_gate"].astype(fp); b_gate = inputs["b_gate"].astype(fp)
    g_out = inputs["g_out"].astype(fp); b_out = inputs["b_out"].astype(fp)

    def blockdiag2(m):   # [64,64] -> [128,128] diag(m, m)
        z = np.zeros((128, 128), m.dtype)
        z[0:64, 0:64] = m
        z[64:128, 64:128] = m
        return z

    w_in = (g_in * s)[:, None] * inputs["w_in"].astype(fp)        # [64o,64i]
    lhsT_in = blockdiag2(np.ascontiguousarray(w_in.T))

    wdw = inputs["w_dw"].astype(fp)[:, 0] * (g_conv * s)[:, None, None]  # [64,3,3]
    idx = np.arange(64)
    lhsT_dw = np.zeros((3, 128, 128), fp)
    lhsT_cx = np.zeros((3, 128, 64), fp)
    for kx in range(3):
        lhsT_dw[kx, idx, idx] = wdw[:, 1, kx]              # row2j   -> out row2j
        lhsT_dw[kx, idx + 64, idx] = wdw[:, 2, kx]         # row2j+1 -> out row2j
        lhsT_dw[kx, idx, idx + 64] = wdw[:, 0, kx]         # row2j   -> out row2j+1
        lhsT_dw[kx, idx + 64, idx + 64] = wdw[:, 1, kx]    # row2j+1 -> out row2j+1
        lhsT_cx[kx, idx, idx] = wdw[:, 2, kx]              # crossB: row2j+2 -> out row2j+1
        lhsT_cx[kx, idx + 64, idx] = wdw[:, 0, kx]         # crossA: row2j-1 -> out row2j

    w_g = (g_gate * s)[:, None] * inputs["w_gate"].astype(fp)     # [128,64]
    lhsT_gl = blockdiag2(np.ascontiguousarray(w_g[0:64].T))
    lhsT_gh = blockdiag2(np.ascontiguousarray(w_g[64:128].T))

    w_out = fp(0.5) * (g_out * s)[:, None] * inputs["w_out"].astype(fp)
    lhsT_out = blockdiag2(np.ascontiguousarray(w_out.T))

    wB_T = np.ascontiguousarray(inputs["wB"].astype(fp).T)        # [64,16]
    lhsT_B = np.concatenate([wB_T, wB_T], axis=0)                  # [128,16]
    lhsT_C = np.ascontiguousarray(inputs["wC"].astype(fp).T)       # [16,64]

    d_ch = inputs["D"].astype(fp)[0, :, 0, 0]                      # [64]
    biases = np.zeros((128, 6), fp)
    biases[:, 0] = np.tile(b_in, 2)
    biases[:, 1] = np.tile(b_conv, 2)
    biases[:, 2] = np.tile(fp(0.5) * b_gate[0:64], 2)
    biases[:, 3] = np.tile(b_gate[64:128], 2)
    biases[:, 4] = np.tile(d_ch, 2)
    biases[:, 5] = np.tile(b_out, 2)

    a_vec = inputs["A"].astype(fp)[0, :, 0]                        # [16]
    k = np.arange(HEAD_L, dtype=fp)
    decay = np.exp(a_vec[:, None] * k[None, :]).astype(fp)         # [16, 2048]
    # the bulk shortcut requires the decay to be EXACT zero past the head
    tail = np.exp(a_vec.astype(fp) * fp(HEAD_L))
    if not np.all(tail == 0.0):
        raise NotImplementedError(
            "decay does not underflow within the head region; enlarge HEAD_PAIRS"
        )

    return {
        "lhsT_in": lhsT_in,
        "lhsT_dw": lhsT_dw.astype(np.float16),
        "lhsT_cx": lhsT_cx.astype(np.float16),
        "lhsT_gl": lhsT_gl.astype(np.float16),
        "lhsT_gh": lhsT_gh.astype(np.float16),
        "lhsT_out": lhsT_out.astype(np.float16),
        "lhsT_B": lhsT_B.astype(np.float16),
        "lhsT_C": lhsT_C,
        "biases": biases,
        "decay": decay,
    }


_prog = None


def _get_prog():
    global _prog
    if _prog is None:
        _prog = _build_program()
    return _prog


def kernel(**inputs):
    consts = _prep_consts(inputs)
    nc = _get_prog()
    x = np.ascontiguousarray(inputs["x"].astype(np.float32))
    in_maps = [dict(consts, x=x[b]) for b in range(B)]
    res = bass_utils.run_bass_kernel_spmd(nc, in_maps, core_ids=list(range(N_CORES)))
    y = np.stack([res.results[b]["y"] for b in range(B)], axis=0)
    return y.astype(np.float32)
